# revision 1
# baseline (speedup 1.0000x reference)
"""CQT (constant-Q transform) + amplitude_to_db kernel for Trainium2.

Full-input contract: kernel(x) takes x [32, 64000] f32 and returns
[32, 84, 126] f32, matching:

    frames = pad(x, n_fft//2)[:, t*HOP + n]          # [B, 126, 16384]
    cr/ci  = frames @ Kr.T / Ki.T                    # [B, 84, 126]
    mag    = sqrt(cr^2 + ci^2)
    out    = amplitude_to_db(mag, ref=max per item, amin=1e-5, top_db=80)

Sharding: pure data parallelism — 4 batch items per NeuronCore on 8 cores.

Per-core compute layout:
  * The frame/filter contraction is one big matmul with K = n_fft = 16384,
    contracted in 128-row chunks. The frame matrix is never materialized:
    padded x stored column-major in SBUF ([128, 628] with x_cm[p,f] =
    xp[f*128+p]) makes chunk c of frames^T a strided AP view
    x_cm[:, c : c+501 : 4] (HOP=512 = 4*128).
  * CQT kernels are ~85% zeros (row k nonzero only in a centered window of
    length L_k, L_k halving per octave). Bins are split into two groups so
    only K-chunks intersecting each group's support are computed:
      group A: bins 0..63  (re+im packed on M: M=128), ~90 chunks
      group B: bins 64..83 (M=64, 32-aligned re/im halves), ~4 chunks
    All 4 items share each matmul via N = 4*126 = 504 <= 512.
  * dB epilogue: m2 = cr^2+ci^2, clamp at amin^2, ACT Ln, per-item max via
    free-dim reduce + GpSimd partition all-reduce, then
    out = max((ln(m2c) - ln(ref2c)) * 10/ln(10), -80).
"""

import os
import numpy as np
import ml_dtypes
from contextlib import ExitStack

import concourse.bass as bass
import concourse.mybir as mybir
from concourse import bacc
from concourse import bass_isa
from concourse.bass_utils import run_bass_kernel_spmd

# matmul input dtype: fp16 keeps the PE at full rate (1 col/cycle) with a
# 10-bit mantissa; bf16 is the fallback; fp32r is the full-precision
# replicated-fp32 PE mode (full rate at N>=256 per the cost model).
MM_DTYPE = os.environ.get("CQT_MM_DTYPE", "fp16")
_DTYPES = {
    "bf16": (mybir.dt.bfloat16, ml_dtypes.bfloat16),
    "fp16": (mybir.dt.float16, np.float16),
    "fp32r": (mybir.dt.float32r, np.float32),
    "fp32": (mybir.dt.float32, np.float32),
}
MM_DT, MM_NP = _DTYPES[MM_DTYPE]

# fp16's normal range bottoms out at 6.1e-5, but low-bin CQT weights peak at
# ~1.7e-6 — pre-scale all weights by 2^14 to clear the subnormal range. The
# scale cancels exactly in the ref-normalized dB output; only the amin clamp
# constant has to be scaled to match.
W_SCALE = 2.0 ** 14 if MM_DTYPE == "fp16" else 1.0

# ---- problem constants (hardcoded; must match the reference) ----
SR = 22050
HOP = 512
N_BINS = 84
BPO = 12
FMIN = 32.70319566257483
AMIN = 1e-5
TOP_DB = 80.0
B = 32
N_SAMP = 64000
N_CORES = 8
NI = B // N_CORES            # items per core = 4
T = 1 + N_SAMP // HOP        # 126 frames
DB_SCALE = 10.0 / np.log(10.0)  # 20*log10(mag) == DB_SCALE * ln(mag^2)

P = 128
SPLIT_BIN = 64               # group A: bins [0,64), group B: bins [64,84)
NB_BINS = N_BINS - SPLIT_BIN  # 20
MB = 64                      # group B stationary width (re at 0:20, im at 32:52)


def _build_cqt_kernels():
    """Same construction as the reference (nnAudio-style direct CQT bank)."""
    Q = 1.0 / (2.0 ** (1.0 / BPO) - 1.0)
    freqs = FMIN * 2.0 ** (np.arange(N_BINS) / BPO)
    lengths = np.ceil(Q * SR / freqs).astype(int)
    n_fft = int(2 ** np.ceil(np.log2(lengths.max())))
    K = np.zeros((N_BINS, n_fft), dtype=np.complex128)
    for k in range(N_BINS):
        L = int(lengths[k])
        t = np.arange(L) - (L - 1) / 2.0
        kern = np.hanning(L) * np.exp(2j * np.pi * freqs[k] * t / SR)
        kern /= np.abs(kern).sum()
        kern /= np.sqrt(L)
        s = (n_fft - L) // 2
        K[k, s:s + L] = kern
    return K.real.astype(np.float32), K.imag.astype(np.float32), n_fft


def _chunk_range(Kr, Ki, bins):
    """Contiguous range of 128-row K-chunks with any nonzero for these bins."""
    nz = (np.abs(Kr[bins]).max(axis=0) + np.abs(Ki[bins]).max(axis=0)) > 0
    idx = np.nonzero(nz)[0]
    return int(idx[0]) // P, int(idx[-1]) // P + 1


Kr, Ki, N_FFT = _build_cqt_kernels()
PAD = N_FFT // 2
FW = (N_SAMP + 2 * PAD) // P      # 628 free-dim width of column-major xp
assert (N_SAMP + 2 * PAD) % P == 0 and HOP == 4 * P

_A0, _A1 = _chunk_range(Kr, Ki, range(0, SPLIT_BIN))
_B0, _B1 = _chunk_range(Kr, Ki, range(SPLIT_BIN, N_BINS))
# Chunks are processed grouped by phase r = c % 4 (ascending), matching the
# order the four phase-DMAs of x land in SBUF, so the first matmuls only
# wait for the first phase slice instead of the whole signal. The four
# B-group chunks are interleaved into the same phase order so they never
# stall the PE FIFO waiting for a late phase.
CHUNKS_A = sorted(range(_A0, _A1), key=lambda c: (c % 4, c))  # 90 chunks
CHUNKS_B = sorted(range(_B0, _B1), key=lambda c: (c % 4, c))  # 4 chunks
NA = len(CHUNKS_A)
NB = len(CHUNKS_B)
# Granular leading weight-DMA pieces: the first matmuls wait on ~130KB of
# weights instead of 0.5MB; later pieces are large to keep issue count low.
SLAB_SIZES = [4, 4, 7, 15, 15, 15, 15, 15]
assert sum(SLAB_SIZES) == NA
SLAB_OFF = [0]
for _sz in SLAB_SIZES:
    SLAB_OFF.append(SLAB_OFF[-1] + _sz)
N_SLABS = len(SLAB_SIZES)


def _slab_of(j):
    for s in range(N_SLABS):
        if j < SLAB_OFF[s + 1]:
            return s, j - SLAB_OFF[s]
    raise IndexError(j)


# One merged, phase-ordered matmul schedule (c % 4 ascending) matching the
# order the x phase-DMAs land; B chunks interleaved so they never stall the
# PE FIFO waiting for a late phase.
SCHEDULE = sorted([("A", c) for c in CHUNKS_A] + [("B", c) for c in CHUNKS_B],
                  key=lambda gc: (gc[1] % 4, gc[1]))


def _pack_weights():
    KrT = Kr.T  # [N_FFT, 84]
    KiT = Ki.T
    wa = np.zeros((P, NA * P), np.float32)
    for j, c in enumerate(CHUNKS_A):
        wa[:, j * P: j * P + SPLIT_BIN] = KrT[c * P:(c + 1) * P, :SPLIT_BIN]
        wa[:, j * P + SPLIT_BIN:(j + 1) * P] = KiT[c * P:(c + 1) * P, :SPLIT_BIN]
    wb = np.zeros((P, NB * MB), np.float32)
    for j, c in enumerate(CHUNKS_B):
        wb[:, j * MB: j * MB + NB_BINS] = KrT[c * P:(c + 1) * P, SPLIT_BIN:]
        wb[:, j * MB + 32: j * MB + 32 + NB_BINS] = KiT[c * P:(c + 1) * P, SPLIT_BIN:]
    return (wa * W_SCALE).astype(MM_NP), (wb * W_SCALE).astype(MM_NP)


WA, WB = _pack_weights()
QW = FW // 4
NT = NI * T


def build_program():
    nc = bacc.Bacc("TRN2", target_bir_lowering=False, debug=False,
                   enable_asserts=True)
    bf16 = MM_DT
    f32 = mybir.dt.float32

    x_in = nc.dram_tensor("x_in", [4, P, NI * QW], bf16, kind="ExternalInput").ap()
    wa_in = nc.dram_tensor("wa_in", [P, NA * P], bf16, kind="ExternalInput").ap()
    wb_in = nc.dram_tensor("wb_in", [P, NB * MB], bf16, kind="ExternalInput").ap()
    out = nc.dram_tensor("out", [N_BINS, NI, T], f32, kind="ExternalOutput").ap()

    xt = nc.alloc_sbuf_tensor("xt", [P, NI * FW], bf16).ap()
    wbt = nc.alloc_sbuf_tensor("wbt", [P, NB * MB], bf16).ap()
    slabs = [nc.alloc_sbuf_tensor(f"wa{s}", [P, SLAB_SIZES[s] * P], bf16).ap()
             for s in range(N_SLABS)]
    junk = nc.alloc_sbuf_tensor("junk", [P, 512], bf16).ap()
    m2 = nc.alloc_sbuf_tensor("m2", [N_BINS, NT], f32).ap()
    tmp = nc.alloc_sbuf_tensor("tmp", [N_BINS, NT], f32).ap()
    r1 = nc.alloc_sbuf_tensor("r1", [N_BINS, NI], f32).ap()
    rall = nc.alloc_sbuf_tensor("rall", [N_BINS, NI], f32).ap()
    lnm = nc.alloc_sbuf_tensor("lnm", [N_BINS, NT], f32).ap()
    lnr = nc.alloc_sbuf_tensor("lnr", [N_BINS, NI], f32).ap()
    db = nc.alloc_sbuf_tensor("db", [N_BINS, NT], f32).ap()
    lnwarm = nc.alloc_sbuf_tensor("lnwarm", [1, 2], f32).ap()

    psW = nc.alloc_psum_tensor("psW", [P, 504], f32).ap()
    psA = nc.alloc_psum_tensor("psA", [P, NT], f32).ap()
    psB = nc.alloc_psum_tensor("psB", [MB, NT], f32).ap()

    # one semaphore per input DMA: per-engine HWDGE round-robins dma_starts
    # over several hardware queues, so completion order on a shared counter
    # is not guaranteed
    s_ph = [nc.alloc_semaphore(f"s_ph{r}") for r in range(4)]
    s_wa = [nc.alloc_semaphore(f"s_wa{s}") for s in range(N_SLABS)]
    s_wb = nc.alloc_semaphore("s_wb")
    s_mi = nc.alloc_semaphore("s_mi")     # junk memset done
    s_pe = nc.alloc_semaphore("s_pe")     # 1 = psB final, 2 = psA final
    s_a = nc.alloc_semaphore("s_a")       # ACT epilogue steps
    s_v = nc.alloc_semaphore("s_v")       # DVE epilogue steps
    s_g2 = nc.alloc_semaphore("s_g2")     # gpsimd all-reduce done
    s_out = nc.alloc_semaphore("s_out")   # output DMA (sync half)
    s_out2 = nc.alloc_semaphore("s_out2")  # output DMA (gpsimd half)

    xv = xt.rearrange("p (r i q) -> p r i q", r=4, i=NI)

    def rhs_for(c):
        r, q0 = c % 4, c // 4
        return xv[:, r, :, q0: q0 + T]

    psAf = psA  # [P, NT]
    psBf = psB
    Ln = mybir.ActivationFunctionType.Ln
    Square = mybir.ActivationFunctionType.Square



    with nc.Block() as block:

        def slab_dma(eng, s):
            eng.dma_start(slabs[s][:],
                          wa_in[:, SLAB_OFF[s] * P:SLAB_OFF[s + 1] * P]
                          ).then_inc(s_wa[s], 16)

        @block.sync
        def _(sync):
            sync.dma_start(xt[:, 0:NI * QW], x_in[0]).then_inc(s_ph[0], 16)
            slab_dma(sync, 0)
            sync.dma_start(xt[:, NI * QW:2 * NI * QW], x_in[1]).then_inc(s_ph[1], 16)
            slab_dma(sync, 3)
            slab_dma(sync, 5)
            slab_dma(sync, 7)
            # output: issue when db is fully written (s_v == 3)
            sync.wait_ge(s_v, 3)
            sync.dma_start(out.rearrange("k i t -> k (i t)")[:, :2 * T],
                           db[:, :2 * T]).then_inc(s_out, 16)
            sync.wait_ge(s_out, 16)

        @block.scalar
        def _(scalar):
            scalar.dma_start(wbt[:], wb_in).then_inc(s_wb, 16)
            slab_dma(scalar, 1)
            scalar.dma_start(xt[:, 2 * NI * QW:3 * NI * QW], x_in[2]
                             ).then_inc(s_ph[2], 16)
            slab_dma(scalar, 4)
            slab_dma(scalar, 6)
            # preload BOTH table slots (Ln set + Square set) while DMAs fly
            scalar.activation(lnwarm[:, 0:1], nc.const_aps.tensor(1.0, (1, 1)), Ln)
            scalar.activation(lnwarm[:, 1:2], nc.const_aps.tensor(1.0, (1, 1)),
                              Square)
            scalar.wait_ge(s_pe, 1)
            scalar.activation(m2[SPLIT_BIN:], psBf[:NB_BINS], Square).then_inc(s_a)
            scalar.activation(tmp[SPLIT_BIN:], psBf[32:32 + NB_BINS], Square
                              ).then_inc(s_a)
            scalar.wait_ge(s_pe, 2)
            scalar.activation(m2[:SPLIT_BIN], psAf[:SPLIT_BIN], Square).then_inc(s_a)
            scalar.activation(tmp[:SPLIT_BIN], psAf[SPLIT_BIN:], Square).then_inc(s_a)
            scalar.wait_ge(s_v, 1)
            scalar.activation(lnm[:], m2[:], Ln).then_inc(s_a)
            scalar.wait_ge(s_g2, 1)
            scalar.activation(lnr[:], rall[:], Ln).then_inc(s_a)

        @block.gpsimd
        def _(gpsimd):
            gpsimd.memset(junk[:], 0.0).then_inc(s_mi, 1)
            slab_dma(gpsimd, 2)
            gpsimd.dma_start(xt[:, 3 * NI * QW:4 * NI * QW], x_in[3]
                             ).then_inc(s_ph[3], 16)
            gpsimd.wait_ge(s_v, 2)
            gpsimd.partition_all_reduce(rall[:], r1[:], channels=N_BINS,
                                        reduce_op=bass_isa.ReduceOp.max
                                        ).then_inc(s_g2, 1)
            gpsimd.wait_ge(s_v, 3)
            gpsimd.dma_start(out.rearrange("k i t -> k (i t)")[:, 2 * T:],
                             db[:, 2 * T:]).then_inc(s_out2, 16)
            gpsimd.wait_ge(s_out2, 16)

        @block.tensor
        def _(tensor):
            tensor.wait_ge(s_mi, 1)
            # tapered warmup: ~3.5us of continuous PE activity ending at the
            # x/weights arrival instant, so HAM is un-throttled (2.4 GHz) for
            # the first real matmul and never lapses in between
            for _ in range(4):
                tensor.matmul(psW[:], lhsT=junk[:, :P], rhs=junk[:, :504],
                              start=True, stop=True)
            for _ in range(3):
                tensor.matmul(psW[:, :252], lhsT=junk[:, :P], rhs=junk[:, :252],
                              start=True, stop=True)
            waited = set()

            def need(sem):
                if id(sem) not in waited:
                    tensor.wait_ge(sem, 16)
                    waited.add(id(sem))

            psBv = psB.rearrange("p (i t) -> p i t", i=NI)
            psAv = psA.rearrange("p (i t) -> p i t", i=NI)
            na_seen = nb_seen = 0
            for grp, c in SCHEDULE:
                need(s_ph[c % 4])
                if grp == "B":
                    j = CHUNKS_B.index(c)
                    need(s_wb)
                    tensor.matmul(psBv[:], lhsT=wbt[:, j * MB:(j + 1) * MB],
                                  rhs=rhs_for(c), start=(nb_seen == 0),
                                  stop=(nb_seen == NB - 1),
                                  skip_group_check=True)
                    nb_seen += 1
                    if nb_seen == NB:
                        tensor.drain().then_inc(s_pe, 1)
                else:
                    j = CHUNKS_A.index(c)
                    s, o = _slab_of(j)
                    need(s_wa[s])
                    tensor.matmul(psAv[:], lhsT=slabs[s][:, o * P:(o + 1) * P],
                                  rhs=rhs_for(c), start=(na_seen == 0),
                                  stop=(na_seen == NA - 1),
                                  skip_group_check=True)
                    na_seen += 1
                    if na_seen == NA:
                        tensor.drain().then_inc(s_pe, 1)

        @block.vector
        def _(vector):
            # drains between dependent same-engine DVE ops: the DVE pipeline
            # gives no RAW forwarding guarantee within one queue
            vector.wait_ge(s_a, 2)
            vector.tensor_add(m2[SPLIT_BIN:], m2[SPLIT_BIN:], tmp[SPLIT_BIN:])
            vector.wait_ge(s_a, 4)
            vector.tensor_add(m2[:SPLIT_BIN], m2[:SPLIT_BIN], tmp[:SPLIT_BIN])
            vector.drain()
            vector.tensor_scalar_max(m2[:], m2[:], float(AMIN * W_SCALE) ** 2)
            vector.drain().then_inc(s_v, 1)
            vector.tensor_reduce(r1[:], m2.rearrange("p (i f) -> p i f", i=NI),
                                 axis=mybir.AxisListType.X, op=mybir.AluOpType.max)
            vector.drain().then_inc(s_v, 1)
            vector.wait_ge(s_a, 6)
            for i in range(NI):
                vector.tensor_scalar(db[:, i * T:(i + 1) * T],
                                     lnm[:, i * T:(i + 1) * T],
                                     lnr[:, i:i + 1], float(DB_SCALE),
                                     mybir.AluOpType.subtract,
                                     mybir.AluOpType.mult)
            vector.drain().then_inc(s_v, 1)

    nc.compile()
    return nc


def pack_x(x):
    """x [B, 64000] f32 -> per-core MM_DTYPE column-major packs [P, NI*FW]."""
    xp = np.pad(np.asarray(x, dtype=np.float32), ((0, 0), (PAD, PAD)))
    # phase-deinterleaved column-major: x_cm[b, p, r, q] = xp[b, (4q+r)*128+p]
    # so chunk c (= 4*q0 + r) streams contiguously in t (HOP = 4*128).
    x_cm = xp.reshape(B, FW // 4, 4, P).transpose(0, 3, 2, 1)  # [B,128,4,157]
    x_cm = x_cm.astype(MM_NP)
    packs = []
    for core in range(N_CORES):
        blk = x_cm[core * NI:(core + 1) * NI]           # [NI, 128, 4, 157]
        packs.append(np.ascontiguousarray(
            blk.transpose(2, 1, 0, 3).reshape(4, P, NI * (FW // 4))))
    return packs


_PROGRAM = None


def _get_program():
    global _PROGRAM
    if _PROGRAM is None:
        _PROGRAM = build_program()
    return _PROGRAM


def run(x, **spmd_kwargs):
    """Run on 8 NeuronCores; returns (output [32, 84, 126] f32, BassKernelResults)."""
    nc = _get_program()
    packs = pack_x(x)
    in_maps = [{"x_in": packs[i], "wa_in": WA, "wb_in": WB}
               for i in range(N_CORES)]
    res = run_bass_kernel_spmd(nc, in_maps, core_ids=list(range(N_CORES)),
                               **spmd_kwargs)
    out = np.concatenate([res.results[i]["out"].transpose(1, 0, 2)
                          for i in range(N_CORES)], axis=0)
    return np.ascontiguousarray(out.astype(np.float32)), res


def kernel(x):
    return run(x)[0]

